# revision 25
# baseline (speedup 1.0000x reference)
"""NT-Xent contrastive loss (forward) on 8 TRN2 NeuronCores via Bass/Tile.

Math: with h = concat(h_i, h_j) [N=8192, D=256], sim = (h @ h.T) / 0.5,
loss = mean_r( logsumexp_j(sim[r, j], j != r) - pos_r ), pos_r = 2 h_i[q].h_j[q].

sim is symmetric, so with a uniform logsumexp shift M the exp'd matrix
E = exp(sim - M) is symmetric too and each unordered block pair is computed
once: a circulant schedule where global row-block R (128 rows) covers the 33
column-blocks at cyclic offsets 0..32.  Offsets 1..31 contribute their row
sums to R's rows (free-axis accumulate) and their column sums (ones.T @ E on
the PE) to the transposed rows; offset 0 is the diagonal block (self-sim
masked by accumulating I.T @ (-1e9 diag) into PSUM); offset 32 is computed
by both endpoints, row-sums only.  Core c owns global row-blocks 8c..8c+7
with all columns pre-rotated by 1024c on the host, so one SPMD program
serves all 8 cores.

The sim blocks are produced by fp8(e4m3) DoubleRow matmuls (K=256 in one
pass, fp32 PSUM accumulate).  Per stripe of 4224 columns: two 1536-column
chunks go to the scalar engine (exact exp, bf16 out, fused row-sum) and
two 512 + one 128 column chunks go to the vector engine as a Schraudolph
u16 bit-trick exp whose bit pattern IS the bf16 value (round(A16*y+B16)),
so the same tile feeds both the row-reduce and the column-sum matmul.
The host assembles S_r from row/column partial sums, takes log in float64,
and computes the positive-pair term directly from the fp32 inputs.
"""

import numpy as np
import ml_dtypes

B = 4096
D = 256
N = 2 * B              # 8192 rows/cols of sim
NCORES = 8
NSTRIPE = 8            # row-blocks (stripes) per core
W = 4224               # stripe width: 33 blocks of 128 (cyclic offsets 0..32)
M_DEFAULT = 161.0      # logsumexp shift; safe while rowmax(2*h@h.T) in [M-70, M+79]
MASK_NEG = -1.0e9

# Schraudolph fast-exp in bf16 bit space: exp(y) ~= bitcast_bf16(round(A16*y+B16)).
A16 = 128.0 / np.log(2.0)
B16 = 16256.0 - 7.446   # offset calibrated to zero the exp-weighted mean error

# ht columns needed per core: [0, 5120), DMA'd in five 1024-col slices of
# one SBUF tile (sub-tile deps let each chunk start on its slice's arrival)
HTW = 5120

# per-stripe chunk layout (offsets relative to stripe start 128j)
#   A1 [0,1536) ACT   D1 [1536,2048) DVE   A2 [2048,3584) ACT
#   D2 [3584,4096) DVE   O3 [4096,4224) DVE (offset-32 block, both sides)
# The three DVE chunks share one u16 bits tile tD[0:512|512:1024|1024:1152]
# reduced by a single 1152-wide row-sum.
# colsum strips (psum row -> relative column range, all within one E tile);
# row 0 must be 512 wide (it carries start=True for the whole psum tile).
CS_STRIPS = [(0, 128, 640), (1, 640, 1152), (2, 1152, 1536), (3, 1536, 2048),
             (4, 3584, 4096), (5, 2048, 2560), (6, 2560, 3072), (7, 3072, 3584)]

TRACE = False
LAST_RESULTS = None

_cache = {}


def _build():
    if "nc" in _cache:
        return _cache["nc"]

    import concourse.tile as tile
    import concourse.mybir as mybir
    from concourse import bacc

    f32 = mybir.dt.float32
    bf16 = mybir.dt.bfloat16
    fp8 = mybir.dt.float8e4
    u16 = mybir.dt.uint16
    DR = mybir.MatmulPerfMode.DoubleRow

    nc = bacc.Bacc("TRN2", target_bir_lowering=False, num_devices=NCORES)
    ht_dram = nc.dram_tensor("ht", [128, 2, HTW], fp8, kind="ExternalInput").ap()
    eye_dram = nc.dram_tensor("eye", [128, 128], bf16, kind="ExternalInput").ap()
    mskd_dram = nc.dram_tensor("mskd", [128, 128], bf16, kind="ExternalInput").ap()
    sel_dram = nc.dram_tensor("sel", [128, 8, 8], bf16, kind="ExternalInput").ap()
    rs_dram = nc.dram_tensor("rs", [128, 3 * NSTRIPE], f32, kind="ExternalOutput").ap()
    cs_dram = nc.dram_tensor("cs", [NSTRIPE, 8, 512], f32, kind="ExternalOutput").ap()

    TS1 = 2.0 * A16                  # psum holds X = a.a ; bits = A16*(2X - M) + B16
    TS2 = B16 - A16 * M_DEFAULT

    with tile.TileContext(nc) as tc:
        with (
            tc.tile_pool(name="hp", bufs=1) as hp,
            tc.tile_pool(name="small", bufs=1) as small,
            tc.tile_pool(name="ep", bufs=2) as ep,
            tc.tile_pool(name="psA", bufs=2, space="PSUM") as psA,
            tc.tile_pool(name="psB", bufs=1, space="PSUM") as psB,
            tc.tile_pool(name="psCS", bufs=1, space="PSUM") as psCS,
        ):

            # per-partition ACT bias (-M) and exp-table warm tile
            biasm_sb = small.tile([128, 1], f32)
            nc.vector.memset(biasm_sb, -M_DEFAULT)
            warm_sb = small.tile([128, 1], f32)
            nc.vector.memset(warm_sb, 0.0)
            nc.scalar.activation(
                out=warm_sb, in_=warm_sb,
                func=mybir.ActivationFunctionType.Exp, bias=0.0, scale=0.0,
            )

            # warm the PE HAM clock gate (~3.4us of activity needed) with
            # N=512 dummy matmuls while the ht DMAs land
            wsrc = small.tile([128, 512], bf16)
            nc.vector.memset(wsrc, 0.0)
            wps = psA.tile([128, 1536], f32, name="psA")
            for w in range(8):
                nc.tensor.matmul(
                    wps[:, (w % 3) * 512:(w % 3) * 512 + 512],
                    lhsT=wsrc[:, 0:128], rhs=wsrc, start=True, stop=True,
                )

            # ht lands in three 2048-col slice DMAs (2KB runs per partition,
            # sub-tile deps let chunks start per slice); the small constants
            # ride the same sync queue between slices, criticality-ordered
            ht_sb = hp.tile([128, 2, HTW], fp8)

            def ht_slice_dma(c0, c1):
                nc.sync.dma_start(out=ht_sb[:, :, c0:c1],
                                  in_=ht_dram[:, :, c0:c1])

            # split the ht stream across two HWDGE queues (sync + scalar)
            # and put the small constants on a third (vector)
            ht_slice_dma(0, 2048)
            mskd_sb = small.tile([128, 128], bf16)
            nc.sync.dma_start(out=mskd_sb, in_=mskd_dram)
            eye_sb = small.tile([128, 128], bf16)
            nc.sync.dma_start(out=eye_sb, in_=eye_dram)
            ht_slice_dma(2048, 4096)
            sel_sb = small.tile([128, 8, 8], bf16)
            nc.sync.dma_start(out=sel_sb, in_=sel_dram)
            ht_slice_dma(4096, 5120)

            def rhs8(x, wdt=512):
                """[128, 2, wdt] fp8 slice of rotated ht at local column x."""
                assert x + wdt <= HTW, (x, wdt)
                return ht_sb[:, :, x:x + wdt]

            res_sb = small.tile([128, 3 * NSTRIPE], f32)

            def mm512(out, base_x, x, wdt=512):
                """Accumulate the K=256 fp8 product into one psum region via
                two K=128 matmuls (FWL hides their weight loads)."""
                for k in range(2):
                    nc.tensor.matmul(
                        out,
                        lhsT=ht_sb[:, k, base_x:base_x + 128],
                        rhs=rhs8(x, wdt)[:, k, :],
                        start=(k == 0), stop=(k == 1),
                    )

            def emit_cs(j, tiles):
                """Column sums of stripe j (one stripe behind the produce)."""
                eA1, eA2, tD = tiles

                def e_slice(lo, hi):
                    if hi <= 1536:
                        return eA1[:, lo:hi]
                    if lo >= 1536 and hi <= 2048:
                        return tD.bitcast(bf16)[:, lo - 1536:hi - 1536]
                    if lo >= 2048 and hi <= 3584:
                        return eA2[:, lo - 2048:hi - 2048]
                    return tD.bitcast(bf16)[:, lo - 3584 + 512:hi - 3584 + 512]

                cstile = psCS.tile([8, 512], f32, name="psCS")
                for k, (row, lo, hi) in enumerate(CS_STRIPS):
                    nc.tensor.matmul(
                        cstile[:, 0:hi - lo],
                        lhsT=sel_sb[:, row, :], rhs=e_slice(lo, hi),
                        start=(k == 0), stop=(k == len(CS_STRIPS) - 1),
                    )
                csb = ep.tile([8, 512], f32, name="csb")
                if j == NSTRIPE - 1:
                    nc.scalar.copy(csb, cstile)
                else:
                    nc.vector.tensor_copy(csb, cstile)
                nc.sync.dma_start(out=cs_dram[j], in_=csb)

            def emit_stripe(j, prev):
                """Produce + exp + row sums for stripe j, with stripe j-1's
                column sums slotted mid-stream; returns E tiles."""
                base = 128 * j
                rcol = 3 * j

                pA1 = psA.tile([128, 1536], f32, name="psA")
                for k in range(2):
                    for cs in range(3):
                        nc.tensor.matmul(
                            pA1[:, cs * 512:(cs + 1) * 512],
                            lhsT=ht_sb[:, k, base:base + 128],
                            rhs=rhs8(base + cs * 512)[:, k, :],
                            start=(k == 0), stop=(k == 1) and (cs != 0),
                        )
                # mask self-similarity: += mskd[m, n] over cols [0,128)
                nc.tensor.matmul(
                    pA1[:, 0:128], lhsT=eye_sb, rhs=mskd_sb,
                    start=False, stop=True,
                )
                eA1 = ep.tile([128, 1536], bf16, name="eA1")
                nc.scalar.activation(
                    out=eA1, in_=pA1, func=mybir.ActivationFunctionType.Exp,
                    bias=biasm_sb, scale=2.0,
                    accum_out=res_sb[:, rcol:rcol + 1],
                )

                tD = ep.tile([128, 1152], u16, name="tD")
                pD1 = psB.tile([128, 512], f32, name="psB")
                mm512(pD1, base, base + 1536)
                nc.vector.tensor_scalar(
                    tD[:, 0:512], pD1, TS1, TS2,
                    mybir.AluOpType.mult, mybir.AluOpType.add)

                pA2 = psA.tile([128, 1536], f32, name="psA")
                for k in range(2):
                    for cs in range(3):
                        nc.tensor.matmul(
                            pA2[:, cs * 512:(cs + 1) * 512],
                            lhsT=ht_sb[:, k, base:base + 128],
                            rhs=rhs8(base + 2048 + cs * 512)[:, k, :],
                            start=(k == 0), stop=(k == 1),
                        )
                eA2 = ep.tile([128, 1536], bf16, name="eA2")
                nc.scalar.activation(
                    out=eA2, in_=pA2, func=mybir.ActivationFunctionType.Exp,
                    bias=biasm_sb, scale=2.0,
                    accum_out=res_sb[:, rcol + 1:rcol + 2],
                )

                if prev is not None:
                    emit_cs(j - 1, prev)

                pD2 = psB.tile([128, 512], f32, name="psB")
                mm512(pD2, base, base + 3584)
                nc.vector.tensor_scalar(
                    tD[:, 512:1024], pD2, TS1, TS2,
                    mybir.AluOpType.mult, mybir.AluOpType.add)
                pO3 = psB.tile([128, 128], f32, name="psB")
                mm512(pO3, base, base + 4096, 128)
                nc.vector.tensor_scalar(
                    tD[:, 1024:1152], pO3, TS1, TS2,
                    mybir.AluOpType.mult, mybir.AluOpType.add)
                nc.vector.reduce_sum(
                    res_sb[:, rcol + 2:rcol + 3], tD.bitcast(bf16),
                    axis=mybir.AxisListType.X)
                return eA1, eA2, tD

            prev = None
            for j in range(NSTRIPE):
                prev = emit_stripe(j, prev)
                if j == NSTRIPE - 1:
                    nc.sync.dma_start(out=rs_dram[:, 0:3 * (NSTRIPE - 1)],
                                      in_=res_sb[:, 0:3 * (NSTRIPE - 1)])
            nc.scalar.dma_start(out=rs_dram[:, 3 * (NSTRIPE - 1):],
                                in_=res_sb[:, 3 * (NSTRIPE - 1):])
            emit_cs(NSTRIPE - 1, prev)

    nc.compile()
    _cache["nc"] = nc
    return nc


def _make_static_inputs(h_i, h_j):
    h = np.concatenate([np.asarray(h_i), np.asarray(h_j)], axis=0).astype(np.float32)
    hT = np.ascontiguousarray(h.T)  # [256, 8192]
    hts = []
    for c in range(NCORES):
        htc = np.roll(hT, -B // 4 * c, axis=1).astype(ml_dtypes.float8_e4m3)
        hts.append({"ht": np.ascontiguousarray(
            htc[:, :HTW].reshape(2, 128, HTW).transpose(1, 0, 2))})
    p = np.arange(128)
    eye = np.zeros((128, 128), dtype=ml_dtypes.bfloat16)
    eye[p, p] = 1.0
    mskd = np.zeros((128, 128), dtype=ml_dtypes.bfloat16)
    mskd[p, p] = MASK_NEG
    sel = np.zeros((128, 8, 8), dtype=ml_dtypes.bfloat16)
    for i in range(8):
        sel[:, i, i] = 1.0
    return hts, eye, mskd, sel


def _assembly_indices():
    """Global-column index map for the colsum strips: [core, stripe, row, 512]."""
    idx = np.zeros((NCORES, NSTRIPE, 8, 512), dtype=np.int64)
    valid = np.zeros((NCORES, NSTRIPE, 8, 512), dtype=np.float64)
    for c in range(NCORES):
        for j in range(NSTRIPE):
            for row, lo, hi in CS_STRIPS:
                w = hi - lo
                loc = 128 * j + lo + np.arange(w)
                idx[c, j, row, :w] = (loc + 1024 * c) % N
                valid[c, j, row, :w] = 1.0
    return idx, valid


_IDX, _VALID = _assembly_indices()


def _axon_reset():
    try:
        import ctypes
        lib = ctypes.CDLL("/opt/axon/libaxon_pjrt.so")
        lib.axon_reset.restype = ctypes.c_int64
        return lib.axon_reset() == 0
    except Exception:
        return False


def _run(nc, hts, eye, mskd, sel):
    global LAST_RESULTS
    from concourse import bass_utils

    in_maps = [
        {**hts[c], "eye": eye, "mskd": mskd, "sel": sel}
        for c in range(NCORES)
    ]
    try:
        results = bass_utils.run_bass_kernel_spmd(
            nc, in_maps, core_ids=list(range(NCORES)), trace=TRACE
        )
    except Exception:
        if not _axon_reset():
            raise
        results = bass_utils.run_bass_kernel_spmd(
            nc, in_maps, core_ids=list(range(NCORES)), trace=TRACE
        )
    LAST_RESULTS = results
    return results.results


def _host_fallback(h_i, h_j):
    """Exact float64 loss on the host (used only if the device result is
    numerically out of range for the fixed logsumexp shift)."""
    h = np.concatenate([np.asarray(h_i), np.asarray(h_j)], 0).astype(np.float64)
    sim = 2.0 * (h @ h.T)
    np.fill_diagonal(sim, -np.inf)
    m = sim.max(1)
    lse = m + np.log(np.exp(sim - m[:, None]).sum(1))
    pos = 2.0 * (h[:B] * h[B:]).sum(1)
    return np.float32((lse - np.concatenate([pos, pos])).mean())


def kernel(h_i, h_j):
    nc = _build()
    hts, eye, mskd, sel = _make_static_inputs(h_i, h_j)
    res = _run(nc, hts, eye, mskd, sel)

    S = np.zeros(N, dtype=np.float64)
    for c in range(NCORES):
        rs = res[c]["rs"].astype(np.float64)            # [128, 24]
        cs = res[c]["cs"].astype(np.float64)            # [8, 8, 512]
        # row sums: stripe j covers global rows 1024c + 128j + p
        rows = (1024 * c + (128 * np.arange(NSTRIPE))[:, None]
                + np.arange(128)[None, :])              # [8, 128]
        S[rows.ravel()] += rs.reshape(128, NSTRIPE, 3).sum(2).T.ravel()
        # column sums
        S += np.bincount(_IDX[c].ravel(),
                         weights=(cs * _VALID[c]).ravel(), minlength=N)

    if not (np.isfinite(S).all() and (S > 0.0).all()):
        return _host_fallback(h_i, h_j)

    lse = M_DEFAULT + np.log(S)
    h_i64 = np.asarray(h_i, dtype=np.float64)
    h_j64 = np.asarray(h_j, dtype=np.float64)
    pos = 2.0 * (h_i64 * h_j64).sum(1)
    loss = lse.mean() - pos.mean()
    return np.array(loss, dtype=np.float32)


if __name__ == "__main__":
    rng = np.random.default_rng(0)
    h_i = rng.standard_normal((B, D), dtype=np.float32)
    h_j = rng.standard_normal((B, D), dtype=np.float32)
    print("loss:", kernel(h_i, h_j))


# revision 26
# speedup vs baseline: 1.1921x; 1.1921x over previous
"""NT-Xent contrastive loss (forward) on 8 TRN2 NeuronCores via Bass/Tile.

Math: with h = concat(h_i, h_j) [N=8192, D=256], sim = (h @ h.T) / 0.5,
loss = mean_r( logsumexp_j(sim[r, j], j != r) - pos_r ), pos_r = 2 h_i[q].h_j[q].

sim is symmetric, so with a uniform logsumexp shift M the exp'd matrix
E = exp(sim - M) is symmetric too and each unordered block pair is computed
once: a circulant schedule where global row-block R (128 rows) covers the 33
column-blocks at cyclic offsets 0..32.  Offsets 1..31 contribute their row
sums to R's rows (free-axis accumulate) and their column sums (ones.T @ E on
the PE) to the transposed rows; offset 0 is the diagonal block (self-sim
masked by accumulating I.T @ (-1e9 diag) into PSUM); offset 32 is computed
by both endpoints, row-sums only.  Core c owns global row-blocks 8c..8c+7
with all columns pre-rotated by 1024c on the host, so one SPMD program
serves all 8 cores.

The sim blocks are produced by fp8(e4m3) DoubleRow matmuls (K=256 in one
pass, fp32 PSUM accumulate).  Per stripe of 4224 columns: two 1536-column
chunks go to the scalar engine (exact exp, bf16 out, fused row-sum) and
two 512 + one 128 column chunks go to the vector engine as a Schraudolph
u16 bit-trick exp whose bit pattern IS the bf16 value (round(A16*y+B16)),
so the same tile feeds both the row-reduce and the column-sum matmul.
The host assembles S_r from row/column partial sums, takes log in float64,
and computes the positive-pair term directly from the fp32 inputs.
"""

import numpy as np
import ml_dtypes

B = 4096
D = 256
N = 2 * B              # 8192 rows/cols of sim
NCORES = 8
NSTRIPE = 8            # row-blocks (stripes) per core
W = 4224               # stripe width: 33 blocks of 128 (cyclic offsets 0..32)
M_DEFAULT = 161.0      # logsumexp shift; safe while rowmax(2*h@h.T) in [M-70, M+79]
MASK_NEG = -1.0e9

# Schraudolph fast-exp in bf16 bit space: exp(y) ~= bitcast_bf16(round(A16*y+B16)).
A16 = 128.0 / np.log(2.0)
B16 = 16256.0 - 7.446   # offset calibrated to zero the exp-weighted mean error

# ht columns needed per core: [0, 5120), DMA'd in five 1024-col slices of
# one SBUF tile (sub-tile deps let each chunk start on its slice's arrival)
HTW = 5120

# per-stripe chunk layout (offsets relative to stripe start 128j)
#   A1 [0,1536) ACT   D1 [1536,2048) DVE   A2 [2048,3584) ACT
#   D2 [3584,4096) DVE   O3 [4096,4224) DVE (offset-32 block, both sides)
# The three DVE chunks share one u16 bits tile tD[0:512|512:1024|1024:1152]
# reduced by a single 1152-wide row-sum.
# colsum strips (psum row -> relative column range, all within one E tile);
# row 0 must be 512 wide (it carries start=True for the whole psum tile).
CS_STRIPS = [(0, 128, 640), (1, 640, 1152), (2, 1152, 1536), (3, 1536, 2048),
             (4, 3584, 4096), (5, 2048, 2560), (6, 2560, 3072), (7, 3072, 3584)]

TRACE = False
LAST_RESULTS = None

_cache = {}


def _build():
    if "nc" in _cache:
        return _cache["nc"]

    import concourse.tile as tile
    import concourse.mybir as mybir
    from concourse import bacc

    f32 = mybir.dt.float32
    bf16 = mybir.dt.bfloat16
    fp8 = mybir.dt.float8e4
    u16 = mybir.dt.uint16
    DR = mybir.MatmulPerfMode.DoubleRow

    nc = bacc.Bacc("TRN2", target_bir_lowering=False, num_devices=NCORES)
    ht_dram = nc.dram_tensor("ht", [128, 2, HTW], fp8, kind="ExternalInput").ap()
    eye_dram = nc.dram_tensor("eye", [128, 128], bf16, kind="ExternalInput").ap()
    mskd_dram = nc.dram_tensor("mskd", [128, 128], bf16, kind="ExternalInput").ap()
    sel_dram = nc.dram_tensor("sel", [128, 8, 8], bf16, kind="ExternalInput").ap()
    rs_dram = nc.dram_tensor("rs", [128, 3 * NSTRIPE], f32, kind="ExternalOutput").ap()
    cs_dram = nc.dram_tensor("cs", [NSTRIPE, 8, 512], f32, kind="ExternalOutput").ap()

    TS1 = 2.0 * A16                  # psum holds X = a.a ; bits = A16*(2X - M) + B16
    TS2 = B16 - A16 * M_DEFAULT

    with tile.TileContext(nc) as tc:
        with (
            tc.tile_pool(name="hp", bufs=1) as hp,
            tc.tile_pool(name="small", bufs=1) as small,
            tc.tile_pool(name="ep", bufs=2) as ep,
            tc.tile_pool(name="psA", bufs=2, space="PSUM") as psA,
            tc.tile_pool(name="psB", bufs=1, space="PSUM") as psB,
            tc.tile_pool(name="psCS", bufs=1, space="PSUM") as psCS,
        ):

            # per-partition ACT bias (-M) and exp-table warm tile
            biasm_sb = small.tile([128, 1], f32)
            nc.vector.memset(biasm_sb, -M_DEFAULT)
            warm_sb = small.tile([128, 1], f32)
            nc.vector.memset(warm_sb, 0.0)
            nc.scalar.activation(
                out=warm_sb, in_=warm_sb,
                func=mybir.ActivationFunctionType.Exp, bias=0.0, scale=0.0,
            )

            # warm the PE HAM clock gate (~3.4us of activity needed) with
            # N=512 dummy matmuls while the ht DMAs land
            wsrc = small.tile([128, 512], bf16)
            nc.vector.memset(wsrc, 0.0)
            wps = psA.tile([128, 1536], f32, name="psA")
            for w in range(8):
                nc.tensor.matmul(
                    wps[:, (w % 3) * 512:(w % 3) * 512 + 512],
                    lhsT=wsrc[:, 0:128], rhs=wsrc, start=True, stop=True,
                )

            # ht lands in three 2048-col slice DMAs (2KB runs per partition,
            # sub-tile deps let chunks start per slice); the small constants
            # ride the same sync queue between slices, criticality-ordered
            ht_sb = hp.tile([128, 2, HTW], fp8)

            def ht_slice_dma(c0, c1):
                nc.sync.dma_start(out=ht_sb[:, :, c0:c1],
                                  in_=ht_dram[:, :, c0:c1])

            # split the ht stream across two HWDGE queues (sync + scalar)
            # and put the small constants on a third (vector)
            ht_slice_dma(0, 2048)
            mskd_sb = small.tile([128, 128], bf16)
            nc.sync.dma_start(out=mskd_sb, in_=mskd_dram)
            eye_sb = small.tile([128, 128], bf16)
            nc.sync.dma_start(out=eye_sb, in_=eye_dram)
            ht_slice_dma(2048, 4096)
            sel_sb = small.tile([128, 8, 8], bf16)
            nc.sync.dma_start(out=sel_sb, in_=sel_dram)
            ht_slice_dma(4096, 5120)

            def rhs8(x, wdt=512):
                """[128, 2, wdt] fp8 slice of rotated ht at local column x."""
                assert x + wdt <= HTW, (x, wdt)
                return ht_sb[:, :, x:x + wdt]

            res_sb = small.tile([128, 3 * NSTRIPE], f32)

            def mm512(out, base_x, x, wdt=512):
                """Accumulate the K=256 fp8 product into one psum region via
                two K=128 matmuls (FWL hides their weight loads)."""
                for k in range(2):
                    nc.tensor.matmul(
                        out,
                        lhsT=ht_sb[:, k, base_x:base_x + 128],
                        rhs=rhs8(x, wdt)[:, k, :],
                        start=(k == 0), stop=(k == 1),
                    )

            def emit_cs(j, tiles):
                """Column sums of stripe j (one stripe behind the produce)."""
                eA1, eA2, tD = tiles

                def e_slice(lo, hi):
                    if hi <= 1536:
                        return eA1[:, lo:hi]
                    if lo >= 1536 and hi <= 2048:
                        return tD.bitcast(bf16)[:, lo - 1536:hi - 1536]
                    if lo >= 2048 and hi <= 3584:
                        return eA2[:, lo - 2048:hi - 2048]
                    return tD.bitcast(bf16)[:, lo - 3584 + 512:hi - 3584 + 512]

                cstile = psCS.tile([8, 512], f32, name="psCS")
                for k, (row, lo, hi) in enumerate(CS_STRIPS):
                    nc.tensor.matmul(
                        cstile[:, 0:hi - lo],
                        lhsT=sel_sb[:, row, :], rhs=e_slice(lo, hi),
                        start=(k == 0), stop=(k == len(CS_STRIPS) - 1),
                    )
                csb = ep.tile([8, 512], f32, name="csb")
                if j == NSTRIPE - 1:
                    nc.scalar.copy(csb, cstile)
                else:
                    nc.vector.tensor_copy(csb, cstile)
                nc.sync.dma_start(out=cs_dram[j], in_=csb)

            def emit_stripe(j, prev):
                """Produce + exp + row sums for stripe j, with stripe j-1's
                column sums slotted mid-stream; returns E tiles."""
                base = 128 * j
                rcol = 3 * j

                pA1 = psA.tile([128, 1536], f32, name="psA")
                for k in range(2):
                    for cs in range(3):
                        nc.tensor.matmul(
                            pA1[:, cs * 512:(cs + 1) * 512],
                            lhsT=ht_sb[:, k, base:base + 128],
                            rhs=rhs8(base + cs * 512)[:, k, :],
                            start=(k == 0), stop=(k == 1) and (cs != 0),
                        )
                # mask self-similarity: += mskd[m, n] over cols [0,128)
                nc.tensor.matmul(
                    pA1[:, 0:128], lhsT=eye_sb, rhs=mskd_sb,
                    start=False, stop=True,
                )
                eA1 = ep.tile([128, 1536], bf16, name="eA1")
                nc.scalar.activation(
                    out=eA1, in_=pA1, func=mybir.ActivationFunctionType.Exp,
                    bias=biasm_sb, scale=2.0,
                    accum_out=res_sb[:, rcol:rcol + 1],
                )

                tD = ep.tile([128, 1152], u16, name="tD")
                pD1 = psB.tile([128, 512], f32, name="psB")
                mm512(pD1, base, base + 1536)
                nc.vector.tensor_scalar(
                    tD[:, 0:512], pD1, TS1, TS2,
                    mybir.AluOpType.mult, mybir.AluOpType.add)

                pA2 = psA.tile([128, 1536], f32, name="psA")
                for k in range(2):
                    for cs in range(3):
                        nc.tensor.matmul(
                            pA2[:, cs * 512:(cs + 1) * 512],
                            lhsT=ht_sb[:, k, base:base + 128],
                            rhs=rhs8(base + 2048 + cs * 512)[:, k, :],
                            start=(k == 0), stop=(k == 1),
                        )
                eA2 = ep.tile([128, 1536], bf16, name="eA2")
                nc.scalar.activation(
                    out=eA2, in_=pA2, func=mybir.ActivationFunctionType.Exp,
                    bias=biasm_sb, scale=2.0,
                    accum_out=res_sb[:, rcol + 1:rcol + 2],
                )

                if prev is not None:
                    emit_cs(j - 1, prev)

                pD2 = psB.tile([128, 512], f32, name="psB")
                mm512(pD2, base, base + 3584)
                nc.vector.tensor_scalar(
                    tD[:, 512:1024], pD2, TS1, TS2,
                    mybir.AluOpType.mult, mybir.AluOpType.add)
                pO3 = psB.tile([128, 128], f32, name="psB")
                mm512(pO3, base, base + 4096, 128)
                nc.vector.tensor_scalar(
                    tD[:, 1024:1152], pO3, TS1, TS2,
                    mybir.AluOpType.mult, mybir.AluOpType.add)
                nc.vector.reduce_sum(
                    res_sb[:, rcol + 2:rcol + 3], tD.bitcast(bf16),
                    axis=mybir.AxisListType.X)
                return eA1, eA2, tD

            prev = None
            for j in range(NSTRIPE):
                prev = emit_stripe(j, prev)
                if j == NSTRIPE - 1:
                    nc.sync.dma_start(out=rs_dram[:, 0:3 * (NSTRIPE - 1)],
                                      in_=res_sb[:, 0:3 * (NSTRIPE - 1)])
            nc.sync.dma_start(out=rs_dram[:, 3 * (NSTRIPE - 1):],
                              in_=res_sb[:, 3 * (NSTRIPE - 1):])
            emit_cs(NSTRIPE - 1, prev)

    nc.compile()
    _cache["nc"] = nc
    return nc


def _make_static_inputs(h_i, h_j):
    h = np.concatenate([np.asarray(h_i), np.asarray(h_j)], axis=0).astype(np.float32)
    hT = np.ascontiguousarray(h.T)  # [256, 8192]
    hts = []
    for c in range(NCORES):
        htc = np.roll(hT, -B // 4 * c, axis=1).astype(ml_dtypes.float8_e4m3)
        hts.append({"ht": np.ascontiguousarray(
            htc[:, :HTW].reshape(2, 128, HTW).transpose(1, 0, 2))})
    p = np.arange(128)
    eye = np.zeros((128, 128), dtype=ml_dtypes.bfloat16)
    eye[p, p] = 1.0
    mskd = np.zeros((128, 128), dtype=ml_dtypes.bfloat16)
    mskd[p, p] = MASK_NEG
    sel = np.zeros((128, 8, 8), dtype=ml_dtypes.bfloat16)
    for i in range(8):
        sel[:, i, i] = 1.0
    return hts, eye, mskd, sel


def _assembly_indices():
    """Global-column index map for the colsum strips: [core, stripe, row, 512]."""
    idx = np.zeros((NCORES, NSTRIPE, 8, 512), dtype=np.int64)
    valid = np.zeros((NCORES, NSTRIPE, 8, 512), dtype=np.float64)
    for c in range(NCORES):
        for j in range(NSTRIPE):
            for row, lo, hi in CS_STRIPS:
                w = hi - lo
                loc = 128 * j + lo + np.arange(w)
                idx[c, j, row, :w] = (loc + 1024 * c) % N
                valid[c, j, row, :w] = 1.0
    return idx, valid


_IDX, _VALID = _assembly_indices()


def _axon_reset():
    try:
        import ctypes
        lib = ctypes.CDLL("/opt/axon/libaxon_pjrt.so")
        lib.axon_reset.restype = ctypes.c_int64
        return lib.axon_reset() == 0
    except Exception:
        return False


def _run(nc, hts, eye, mskd, sel):
    global LAST_RESULTS
    from concourse import bass_utils

    in_maps = [
        {**hts[c], "eye": eye, "mskd": mskd, "sel": sel}
        for c in range(NCORES)
    ]
    try:
        results = bass_utils.run_bass_kernel_spmd(
            nc, in_maps, core_ids=list(range(NCORES)), trace=TRACE
        )
    except Exception:
        if not _axon_reset():
            raise
        results = bass_utils.run_bass_kernel_spmd(
            nc, in_maps, core_ids=list(range(NCORES)), trace=TRACE
        )
    LAST_RESULTS = results
    return results.results


def _host_fallback(h_i, h_j):
    """Exact float64 loss on the host (used only if the device result is
    numerically out of range for the fixed logsumexp shift)."""
    h = np.concatenate([np.asarray(h_i), np.asarray(h_j)], 0).astype(np.float64)
    sim = 2.0 * (h @ h.T)
    np.fill_diagonal(sim, -np.inf)
    m = sim.max(1)
    lse = m + np.log(np.exp(sim - m[:, None]).sum(1))
    pos = 2.0 * (h[:B] * h[B:]).sum(1)
    return np.float32((lse - np.concatenate([pos, pos])).mean())


def kernel(h_i, h_j):
    nc = _build()
    hts, eye, mskd, sel = _make_static_inputs(h_i, h_j)
    res = _run(nc, hts, eye, mskd, sel)

    S = np.zeros(N, dtype=np.float64)
    for c in range(NCORES):
        rs = res[c]["rs"].astype(np.float64)            # [128, 24]
        cs = res[c]["cs"].astype(np.float64)            # [8, 8, 512]
        # row sums: stripe j covers global rows 1024c + 128j + p
        rows = (1024 * c + (128 * np.arange(NSTRIPE))[:, None]
                + np.arange(128)[None, :])              # [8, 128]
        S[rows.ravel()] += rs.reshape(128, NSTRIPE, 3).sum(2).T.ravel()
        # column sums
        S += np.bincount(_IDX[c].ravel(),
                         weights=(cs * _VALID[c]).ravel(), minlength=N)

    if not (np.isfinite(S).all() and (S > 0.0).all()):
        return _host_fallback(h_i, h_j)

    lse = M_DEFAULT + np.log(S)
    h_i64 = np.asarray(h_i, dtype=np.float64)
    h_j64 = np.asarray(h_j, dtype=np.float64)
    pos = 2.0 * (h_i64 * h_j64).sum(1)
    loss = lse.mean() - pos.mean()
    return np.array(loss, dtype=np.float32)


if __name__ == "__main__":
    rng = np.random.default_rng(0)
    h_i = rng.standard_normal((B, D), dtype=np.float32)
    h_j = rng.standard_normal((B, D), dtype=np.float32)
    print("loss:", kernel(h_i, h_j))


# revision 27
# speedup vs baseline: 1.2041x; 1.0101x over previous
"""NT-Xent contrastive loss (forward) on 8 TRN2 NeuronCores via Bass/Tile.

Math: with h = concat(h_i, h_j) [N=8192, D=256], sim = (h @ h.T) / 0.5,
loss = mean_r( logsumexp_j(sim[r, j], j != r) - pos_r ), pos_r = 2 h_i[q].h_j[q].

sim is symmetric, so with a uniform logsumexp shift M the exp'd matrix
E = exp(sim - M) is symmetric too and each unordered block pair is computed
once: a circulant schedule where global row-block R (128 rows) covers the 33
column-blocks at cyclic offsets 0..32.  Offsets 1..31 contribute their row
sums to R's rows (free-axis accumulate) and their column sums (ones.T @ E on
the PE) to the transposed rows; offset 0 is the diagonal block (self-sim
masked by accumulating I.T @ (-1e9 diag) into PSUM); offset 32 is computed
by both endpoints, row-sums only.  Core c owns global row-blocks 8c..8c+7
with all columns pre-rotated by 1024c on the host, so one SPMD program
serves all 8 cores.

The sim blocks are produced by fp8(e4m3) DoubleRow matmuls (K=256 in one
pass, fp32 PSUM accumulate).  Per stripe of 4224 columns: two 1536-column
chunks go to the scalar engine (exact exp, bf16 out, fused row-sum) and
two 512 + one 128 column chunks go to the vector engine as a Schraudolph
u16 bit-trick exp whose bit pattern IS the bf16 value (round(A16*y+B16)),
so the same tile feeds both the row-reduce and the column-sum matmul.
The host assembles S_r from row/column partial sums, takes log in float64,
and computes the positive-pair term directly from the fp32 inputs.
"""

import numpy as np
import ml_dtypes

B = 4096
D = 256
N = 2 * B              # 8192 rows/cols of sim
NCORES = 8
NSTRIPE = 8            # row-blocks (stripes) per core
W = 4224               # stripe width: 33 blocks of 128 (cyclic offsets 0..32)
M_DEFAULT = 161.0      # logsumexp shift; safe while rowmax(2*h@h.T) in [M-70, M+79]
MASK_NEG = -1.0e9

# Schraudolph fast-exp in bf16 bit space: exp(y) ~= bitcast_bf16(round(A16*y+B16)).
A16 = 128.0 / np.log(2.0)
B16 = 16256.0 - 7.446   # offset calibrated to zero the exp-weighted mean error

# ht columns needed per core: [0, 5120), DMA'd in five 1024-col slices of
# one SBUF tile (sub-tile deps let each chunk start on its slice's arrival)
HTW = 5120

# per-stripe chunk layout (offsets relative to stripe start 128j)
#   A1 [0,1536) ACT   D1 [1536,2048) DVE   A2 [2048,3584) ACT
#   D2 [3584,4096) DVE   O3 [4096,4224) DVE (offset-32 block, both sides)
# The three DVE chunks share one u16 bits tile tD[0:512|512:1024|1024:1152]
# reduced by a single 1152-wide row-sum.
# colsum strips (psum row -> relative column range, all within one E tile);
# row 0 must be 512 wide (it carries start=True for the whole psum tile).
CS_STRIPS = [(0, 128, 640), (1, 640, 1152), (2, 1152, 1536), (3, 1536, 2048),
             (4, 3584, 4096), (5, 2048, 2560), (6, 2560, 3072), (7, 3072, 3584)]

TRACE = False
LAST_RESULTS = None

_cache = {}


def _build():
    if "nc" in _cache:
        return _cache["nc"]

    import concourse.tile as tile
    import concourse.mybir as mybir
    from concourse import bacc

    f32 = mybir.dt.float32
    bf16 = mybir.dt.bfloat16
    fp8 = mybir.dt.float8e4
    u16 = mybir.dt.uint16
    DR = mybir.MatmulPerfMode.DoubleRow

    nc = bacc.Bacc("TRN2", target_bir_lowering=False, num_devices=NCORES)
    ht_dram = nc.dram_tensor("ht", [128, 2, HTW], fp8, kind="ExternalInput").ap()
    eye_dram = nc.dram_tensor("eye", [128, 128], bf16, kind="ExternalInput").ap()
    mskd_dram = nc.dram_tensor("mskd", [128, 128], bf16, kind="ExternalInput").ap()
    sel_dram = nc.dram_tensor("sel", [128, 8, 8], bf16, kind="ExternalInput").ap()
    rs_dram = nc.dram_tensor("rs", [128, 3 * NSTRIPE], f32, kind="ExternalOutput").ap()
    cs_dram = nc.dram_tensor("cs", [NSTRIPE, 8, 512], f32, kind="ExternalOutput").ap()

    TS1 = 2.0 * A16                  # psum holds X = a.a ; bits = A16*(2X - M) + B16
    TS2 = B16 - A16 * M_DEFAULT

    with tile.TileContext(nc) as tc:
        with (
            tc.tile_pool(name="hp", bufs=1) as hp,
            tc.tile_pool(name="small", bufs=1) as small,
            tc.tile_pool(name="ep", bufs=2) as ep,
            tc.tile_pool(name="psA", bufs=2, space="PSUM") as psA,
            tc.tile_pool(name="psB", bufs=1, space="PSUM") as psB,
            tc.tile_pool(name="psCS", bufs=1, space="PSUM") as psCS,
        ):

            # per-partition ACT bias (-M) and exp-table warm tile
            biasm_sb = small.tile([128, 1], f32)
            nc.vector.memset(biasm_sb, -M_DEFAULT)
            warm_sb = small.tile([128, 1], f32)
            nc.vector.memset(warm_sb, 0.0)
            nc.scalar.activation(
                out=warm_sb, in_=warm_sb,
                func=mybir.ActivationFunctionType.Exp, bias=0.0, scale=0.0,
            )

            # warm the PE HAM clock gate (~3.4us of activity needed) with
            # N=512 dummy matmuls while the ht DMAs land
            wsrc = small.tile([128, 512], bf16)
            nc.vector.memset(wsrc, 0.0)
            wps = psA.tile([128, 1536], f32, name="psA")
            for w in range(8):
                nc.tensor.matmul(
                    wps[:, (w % 3) * 512:(w % 3) * 512 + 512],
                    lhsT=wsrc[:, 0:128], rhs=wsrc, start=True, stop=True,
                )

            # ht lands in three 2048-col slice DMAs (2KB runs per partition,
            # sub-tile deps let chunks start per slice); the small constants
            # ride the same sync queue between slices, criticality-ordered
            ht_sb = hp.tile([128, 2, HTW], fp8)

            def ht_slice_dma(c0, c1):
                nc.sync.dma_start(out=ht_sb[:, :, c0:c1],
                                  in_=ht_dram[:, :, c0:c1])

            # split the ht stream across two HWDGE queues (sync + scalar)
            # and put the small constants on a third (vector)
            ht_slice_dma(0, 2048)
            ht_slice_dma(2048, 4096)
            mskd_sb = small.tile([128, 128], bf16)
            nc.sync.dma_start(out=mskd_sb, in_=mskd_dram)
            eye_sb = small.tile([128, 128], bf16)
            nc.sync.dma_start(out=eye_sb, in_=eye_dram)
            sel_sb = small.tile([128, 8, 8], bf16)
            nc.sync.dma_start(out=sel_sb, in_=sel_dram)
            ht_slice_dma(4096, 5120)

            def rhs8(x, wdt=512):
                """[128, 2, wdt] fp8 slice of rotated ht at local column x."""
                assert x + wdt <= HTW, (x, wdt)
                return ht_sb[:, :, x:x + wdt]

            res_sb = small.tile([128, 3 * NSTRIPE], f32)

            def mm512(out, base_x, x, wdt=512):
                """Accumulate the K=256 fp8 product into one psum region via
                two K=128 matmuls (FWL hides their weight loads)."""
                for k in range(2):
                    nc.tensor.matmul(
                        out,
                        lhsT=ht_sb[:, k, base_x:base_x + 128],
                        rhs=rhs8(x, wdt)[:, k, :],
                        start=(k == 0), stop=(k == 1),
                    )

            def emit_cs(j, tiles):
                """Column sums of stripe j (one stripe behind the produce)."""
                eA1, eA2, tD = tiles

                def e_slice(lo, hi):
                    if hi <= 1536:
                        return eA1[:, lo:hi]
                    if lo >= 1536 and hi <= 2048:
                        return tD.bitcast(bf16)[:, lo - 1536:hi - 1536]
                    if lo >= 2048 and hi <= 3584:
                        return eA2[:, lo - 2048:hi - 2048]
                    return tD.bitcast(bf16)[:, lo - 3584 + 512:hi - 3584 + 512]

                cstile = psCS.tile([8, 512], f32, name="psCS")
                for k, (row, lo, hi) in enumerate(CS_STRIPS):
                    nc.tensor.matmul(
                        cstile[:, 0:hi - lo],
                        lhsT=sel_sb[:, row, :], rhs=e_slice(lo, hi),
                        start=(k == 0), stop=(k == len(CS_STRIPS) - 1),
                    )
                csb = ep.tile([8, 512], f32, name="csb")
                if j == NSTRIPE - 1:
                    nc.scalar.copy(csb, cstile)
                else:
                    nc.vector.tensor_copy(csb, cstile)
                nc.sync.dma_start(out=cs_dram[j], in_=csb)

            def emit_stripe(j, prev):
                """Produce + exp + row sums for stripe j, with stripe j-1's
                column sums slotted mid-stream; returns E tiles."""
                base = 128 * j
                rcol = 3 * j

                pA1 = psA.tile([128, 1536], f32, name="psA")
                for k in range(2):
                    for cs in range(3):
                        nc.tensor.matmul(
                            pA1[:, cs * 512:(cs + 1) * 512],
                            lhsT=ht_sb[:, k, base:base + 128],
                            rhs=rhs8(base + cs * 512)[:, k, :],
                            start=(k == 0), stop=(k == 1) and (cs != 0),
                        )
                # mask self-similarity: += mskd[m, n] over cols [0,128)
                nc.tensor.matmul(
                    pA1[:, 0:128], lhsT=eye_sb, rhs=mskd_sb,
                    start=False, stop=True,
                )
                eA1 = ep.tile([128, 1536], bf16, name="eA1")
                nc.scalar.activation(
                    out=eA1, in_=pA1, func=mybir.ActivationFunctionType.Exp,
                    bias=biasm_sb, scale=2.0,
                    accum_out=res_sb[:, rcol:rcol + 1],
                )

                tD = ep.tile([128, 1152], u16, name="tD")
                pD1 = psB.tile([128, 512], f32, name="psB")
                mm512(pD1, base, base + 1536)
                nc.vector.tensor_scalar(
                    tD[:, 0:512], pD1, TS1, TS2,
                    mybir.AluOpType.mult, mybir.AluOpType.add)

                pA2 = psA.tile([128, 1536], f32, name="psA")
                for k in range(2):
                    for cs in range(3):
                        nc.tensor.matmul(
                            pA2[:, cs * 512:(cs + 1) * 512],
                            lhsT=ht_sb[:, k, base:base + 128],
                            rhs=rhs8(base + 2048 + cs * 512)[:, k, :],
                            start=(k == 0), stop=(k == 1),
                        )
                eA2 = ep.tile([128, 1536], bf16, name="eA2")
                nc.scalar.activation(
                    out=eA2, in_=pA2, func=mybir.ActivationFunctionType.Exp,
                    bias=biasm_sb, scale=2.0,
                    accum_out=res_sb[:, rcol + 1:rcol + 2],
                )

                if prev is not None:
                    emit_cs(j - 1, prev)

                pD2 = psB.tile([128, 512], f32, name="psB")
                mm512(pD2, base, base + 3584)
                nc.vector.tensor_scalar(
                    tD[:, 512:1024], pD2, TS1, TS2,
                    mybir.AluOpType.mult, mybir.AluOpType.add)
                pO3 = psB.tile([128, 128], f32, name="psB")
                mm512(pO3, base, base + 4096, 128)
                nc.vector.tensor_scalar(
                    tD[:, 1024:1152], pO3, TS1, TS2,
                    mybir.AluOpType.mult, mybir.AluOpType.add)
                nc.vector.reduce_sum(
                    res_sb[:, rcol + 2:rcol + 3], tD.bitcast(bf16),
                    axis=mybir.AxisListType.X)
                return eA1, eA2, tD

            prev = None
            for j in range(NSTRIPE):
                prev = emit_stripe(j, prev)
                if j == NSTRIPE - 1:
                    nc.sync.dma_start(out=rs_dram[:, 0:3 * (NSTRIPE - 1)],
                                      in_=res_sb[:, 0:3 * (NSTRIPE - 1)])
            nc.sync.dma_start(out=rs_dram[:, 3 * (NSTRIPE - 1):],
                              in_=res_sb[:, 3 * (NSTRIPE - 1):])
            emit_cs(NSTRIPE - 1, prev)

    nc.compile()
    _cache["nc"] = nc
    return nc


def _make_static_inputs(h_i, h_j):
    h = np.concatenate([np.asarray(h_i), np.asarray(h_j)], axis=0).astype(np.float32)
    hT = np.ascontiguousarray(h.T)  # [256, 8192]
    hts = []
    for c in range(NCORES):
        htc = np.roll(hT, -B // 4 * c, axis=1).astype(ml_dtypes.float8_e4m3)
        hts.append({"ht": np.ascontiguousarray(
            htc[:, :HTW].reshape(2, 128, HTW).transpose(1, 0, 2))})
    p = np.arange(128)
    eye = np.zeros((128, 128), dtype=ml_dtypes.bfloat16)
    eye[p, p] = 1.0
    mskd = np.zeros((128, 128), dtype=ml_dtypes.bfloat16)
    mskd[p, p] = MASK_NEG
    sel = np.zeros((128, 8, 8), dtype=ml_dtypes.bfloat16)
    for i in range(8):
        sel[:, i, i] = 1.0
    return hts, eye, mskd, sel


def _assembly_indices():
    """Global-column index map for the colsum strips: [core, stripe, row, 512]."""
    idx = np.zeros((NCORES, NSTRIPE, 8, 512), dtype=np.int64)
    valid = np.zeros((NCORES, NSTRIPE, 8, 512), dtype=np.float64)
    for c in range(NCORES):
        for j in range(NSTRIPE):
            for row, lo, hi in CS_STRIPS:
                w = hi - lo
                loc = 128 * j + lo + np.arange(w)
                idx[c, j, row, :w] = (loc + 1024 * c) % N
                valid[c, j, row, :w] = 1.0
    return idx, valid


_IDX, _VALID = _assembly_indices()


def _axon_reset():
    try:
        import ctypes
        lib = ctypes.CDLL("/opt/axon/libaxon_pjrt.so")
        lib.axon_reset.restype = ctypes.c_int64
        return lib.axon_reset() == 0
    except Exception:
        return False


def _run(nc, hts, eye, mskd, sel):
    global LAST_RESULTS
    from concourse import bass_utils

    in_maps = [
        {**hts[c], "eye": eye, "mskd": mskd, "sel": sel}
        for c in range(NCORES)
    ]
    try:
        results = bass_utils.run_bass_kernel_spmd(
            nc, in_maps, core_ids=list(range(NCORES)), trace=TRACE
        )
    except Exception:
        if not _axon_reset():
            raise
        results = bass_utils.run_bass_kernel_spmd(
            nc, in_maps, core_ids=list(range(NCORES)), trace=TRACE
        )
    LAST_RESULTS = results
    return results.results


def _host_fallback(h_i, h_j):
    """Exact float64 loss on the host (used only if the device result is
    numerically out of range for the fixed logsumexp shift)."""
    h = np.concatenate([np.asarray(h_i), np.asarray(h_j)], 0).astype(np.float64)
    sim = 2.0 * (h @ h.T)
    np.fill_diagonal(sim, -np.inf)
    m = sim.max(1)
    lse = m + np.log(np.exp(sim - m[:, None]).sum(1))
    pos = 2.0 * (h[:B] * h[B:]).sum(1)
    return np.float32((lse - np.concatenate([pos, pos])).mean())


def kernel(h_i, h_j):
    nc = _build()
    hts, eye, mskd, sel = _make_static_inputs(h_i, h_j)
    res = _run(nc, hts, eye, mskd, sel)

    S = np.zeros(N, dtype=np.float64)
    for c in range(NCORES):
        rs = res[c]["rs"].astype(np.float64)            # [128, 24]
        cs = res[c]["cs"].astype(np.float64)            # [8, 8, 512]
        # row sums: stripe j covers global rows 1024c + 128j + p
        rows = (1024 * c + (128 * np.arange(NSTRIPE))[:, None]
                + np.arange(128)[None, :])              # [8, 128]
        S[rows.ravel()] += rs.reshape(128, NSTRIPE, 3).sum(2).T.ravel()
        # column sums
        S += np.bincount(_IDX[c].ravel(),
                         weights=(cs * _VALID[c]).ravel(), minlength=N)

    if not (np.isfinite(S).all() and (S > 0.0).all()):
        return _host_fallback(h_i, h_j)

    lse = M_DEFAULT + np.log(S)
    h_i64 = np.asarray(h_i, dtype=np.float64)
    h_j64 = np.asarray(h_j, dtype=np.float64)
    pos = 2.0 * (h_i64 * h_j64).sum(1)
    loss = lse.mean() - pos.mean()
    return np.array(loss, dtype=np.float32)


if __name__ == "__main__":
    rng = np.random.default_rng(0)
    h_i = rng.standard_normal((B, D), dtype=np.float32)
    h_j = rng.standard_normal((B, D), dtype=np.float32)
    print("loss:", kernel(h_i, h_j))
